# revision 1
# baseline (speedup 1.0000x reference)
"""Trainium2 Bass kernel for nn_HarmonicOscillatorOrbitals.

out[b, i, j] = exp(-s^2/2) * H_j(s), s = omega * x[b, i, 0], j = 0..31
(physicists' Hermite polynomials), data-parallel over 8 NeuronCores on
the leading batch axis.

Per core (8192 batches = 262144 scalars as [128 partitions, E=2048]):
  t   = 2*omega*x
  env = exp(-s^2/2) = 2^(t^2 * -log2(e)/8), computed exactly on DVE:
        2^n by float-magic + integer exponent shift, 2^f by a degree-5
        polynomial (fused scalar_tensor_tensor Horner chain) — the ACT
        spline Exp is ~1e-5 off, this path is ~1e-6.
  G_0 = env, G_1 = t*env, G_k = t*G_{k-1} - 2(k-1)*G_{k-2}  (= env*H_k)

The per-element recurrence is serial in k, so elements are split into
two independent column slices: DVE runs one chain (tensor_mul + fused
scalar_tensor_tensor), GPSIMD the other (tensor_mul + tensor_sub, with
ACT supplying the 2(k-1)*G_{k-2} scale-copies two steps ahead). Each
chain keeps its engine fully busy with no cross-engine ping-pong.

G_k slices stay contiguous in SBUF; DRAM output is k-major
[128, 32, E] (1.3KB DMA descriptors) and the host permutes to
(batch, i, j) while unsharding.
"""

from contextlib import ExitStack

import numpy as np

import concourse.bacc as bacc
import concourse.mybir as mybir
import concourse.tile as tile
from concourse.bass_utils import run_bass_kernel_spmd

F32 = mybir.dt.float32
I32 = mybir.dt.int32
AF = mybir.ActivationFunctionType
ALU = mybir.AluOpType

NJ = 32          # number of Hermite orders
N_CORES = 8
B = 65536        # full batch
BC = B // N_CORES
E = BC * NJ // 128   # 2048 free elems per partition per core

TILE_F = 512     # columns per tile
FD = 300         # DVE-owned columns per tile (rest on GPSIMD)

# exp2: env = 2^v, v = sq * K4 with sq = t^2 = 4 s^2
K4 = float(np.float32(-np.log2(np.e) / 8.0))
MAGIC = float(np.float32(1.5 * 2**23))
EXP_B4, EXP_B3, EXP_B2, EXP_B1 = 7.292242, 41.85769, 181.15059, 522.6992
EXP_A5, EXP_A0 = 0.0013260915, 1.0
EXP_EIMM = 127 - 0x4B400000  # (bits(w) + EXP_EIMM) << 23 == bits(2^n)


def _build(e=E, tile_f=TILE_F, fd=FD, accurate_env=False):
    nc = bacc.Bacc("TRN2", target_bir_lowering=False, debug=False)
    x_d = nc.dram_tensor("x", [128, e], F32, kind="ExternalInput").ap()
    om_d = nc.dram_tensor("om", [1, 1], F32, kind="ExternalInput").ap()
    # raw tile dump: per f-tile, the DVE-slice tile [128, NJ*fd] then the
    # GPSIMD-slice tile [128, NJ*fg], verbatim — host unscrambles
    out_d = nc.dram_tensor("out", [128, NJ * e], F32, kind="ExternalOutput").ap()

    fg = tile_f - fd
    n_tiles = e // tile_f
    with tile.TileContext(nc) as tc, ExitStack() as ctx:
        cpool = ctx.enter_context(tc.tile_pool(name="const", bufs=1))
        xp = ctx.enter_context(tc.tile_pool(name="xp", bufs=4))
        ep = ctx.enter_context(tc.tile_pool(name="ep", bufs=3))
        qd = ctx.enter_context(tc.tile_pool(name="qd", bufs=4))
        qg = ctx.enter_context(tc.tile_pool(name="qg", bufs=4))
        gdp = ctx.enter_context(tc.tile_pool(name="gdp", bufs=2))
        ggp = ctx.enter_context(tc.tile_pool(name="ggp", bufs=2))

        om1 = cpool.tile([128, 1], F32)
        nc.sync.dma_start(om1[0:1, :], om_d[:, :])
        om2 = cpool.tile([128, 1], F32)
        nc.gpsimd.partition_broadcast(om2[:, :], om1[0:1, :])
        nc.scalar.mul(om2[:, :], om2[:, :], 2.0)  # om2 = 2*omega

        # int32 constants for the exponent fixup: (bits(w) + EXP_EIMM) << 23
        addc = cpool.tile([128, tile_f], I32)
        nc.vector.memset(addc[:, :], EXP_EIMM)
        t23 = cpool.tile([128, tile_f], I32)
        nc.vector.memset(t23[:, :], 23)

        for it in range(n_tiles):
            f0 = it * tile_f
            x_t = xp.tile([128, tile_f], F32)
            nc.sync.dma_start(x_t[:, :], x_d[:, f0 : f0 + tile_f])
            t_t = xp.tile([128, tile_f], F32, tag="t")
            nc.scalar.mul(t_t[:, :], x_t[:, :], om2[:, 0:1])  # t = 2*omega*x

            # four k-quarters per slice: DMA each out as soon as its 8
            # columns are done, so pool slots recycle at 1/4-tile grain
            gd_q = [
                gdp.tile([128, 8 * fd], F32, name=f"gdq{q}_{it}", tag=f"gd{q}")
                for q in range(4)
            ]
            gg_q = [
                ggp.tile([128, 8 * fg], F32, name=f"ggq{q}_{it}", tag=f"gg{q}")
                for q in range(4)
            ]

            def gds(k):
                return gd_q[k // 8][:, (k % 8) * fd : (k % 8 + 1) * fd]

            def ggs(k):
                return gg_q[k // 8][:, (k % 8) * fg : (k % 8 + 1) * fg]

            base = it * NJ * tile_f

            def flush_quarter(q):
                nc.sync.dma_start(
                    out_d[:, base + q * 8 * fd : base + (q + 1) * 8 * fd],
                    gd_q[q][:, :],
                )
                goff = base + NJ * fd
                nc.sync.dma_start(
                    out_d[:, goff + q * 8 * fg : goff + (q + 1) * 8 * fg],
                    gg_q[q][:, :],
                )

            if accurate_env:
                # ---- exact exp2 on DVE, full tile width ----
                sq = ep.tile([128, tile_f], F32, tag="sq")
                nc.vector.tensor_mul(sq[:, :], t_t[:, :], t_t[:, :])
                v_t = ep.tile([128, tile_f], F32, tag="v")
                nc.vector.tensor_scalar_mul(v_t[:, :], sq[:, :], K4)
                w_t = ep.tile([128, tile_f], F32, tag="w")
                nc.vector.tensor_scalar_add(w_t[:, :], v_t[:, :], MAGIC)
                n_t = ep.tile([128, tile_f], F32, tag="n")
                nc.vector.tensor_scalar_sub(n_t[:, :], w_t[:, :], MAGIC)
                f_t = ep.tile([128, tile_f], F32, tag="f")
                nc.vector.tensor_sub(f_t[:, :], v_t[:, :], n_t[:, :])
                p_t = ep.tile([128, tile_f], F32, tag="p")
                nc.vector.scalar_tensor_tensor(
                    p_t[:, :], f_t[:, :], EXP_B4, f_t[:, :], ALU.add, ALU.mult
                )
                for bb in (EXP_B3, EXP_B2, EXP_B1):
                    nc.vector.scalar_tensor_tensor(
                        p_t[:, :], p_t[:, :], bb, f_t[:, :], ALU.add, ALU.mult
                    )
                nc.vector.tensor_scalar(
                    p_t[:, :], p_t[:, :], EXP_A5, EXP_A0, ALU.mult, ALU.add
                )
                e2_t = ep.tile([128, tile_f], I32, tag="e2")
                nc.vector.tensor_tensor(
                    e2_t[:, :], w_t[:, :].bitcast(I32), addc[:, :], ALU.add
                )
                nc.vector.tensor_tensor(
                    e2_t[:, :], e2_t[:, :], t23[:, :], ALU.logical_shift_left
                )
                e2f = e2_t[:, :].bitcast(F32)
                # env split straight into the two G tiles (k = 0)
                nc.vector.tensor_mul(gds(0), p_t[:, 0:fd], e2f[:, 0:fd])
                nc.vector.tensor_mul(ggs(0), p_t[:, fd:], e2f[:, fd:])
            else:
                sq = ep.tile([128, tile_f], F32, tag="sq")
                nc.scalar.activation(sq[:, :], t_t[:, :], AF.Square, scale=0.5)
                nc.scalar.activation(gds(0), sq[:, 0:fd], AF.Exp, scale=-0.5)
                nc.scalar.activation(ggs(0), sq[:, fd:], AF.Exp, scale=-0.5)

            # G_1 = t * env, each engine seeds its own chain
            nc.vector.tensor_mul(gds(1), t_t[:, 0:fd], gds(0))
            nc.gpsimd.tensor_mul(ggs(1), t_t[:, fd:], ggs(0))

            for k in range(2, NJ):
                c = 2.0 * (k - 1)
                # DVE chain
                q_t = qd.tile([128, fd], F32)
                nc.vector.tensor_mul(q_t[:, :], t_t[:, 0:fd], gds(k - 1))
                nc.vector.scalar_tensor_tensor(
                    gds(k), gds(k - 2), -c, q_t[:, :], ALU.mult, ALU.add
                )
                # GPSIMD chain (ACT supplies c*G_{k-2})
                qg_t = qg.tile([128, fg], F32)
                rg_t = qg.tile([128, fg], F32, tag="rg")
                nc.gpsimd.tensor_mul(qg_t[:, :], t_t[:, fd:], ggs(k - 1))
                nc.scalar.mul(rg_t[:, :], ggs(k - 2), c)
                nc.gpsimd.tensor_sub(ggs(k), qg_t[:, :], rg_t[:, :])
                if k % 8 == 7:
                    flush_quarter(k // 8)

    nc.compile()
    return nc


_CACHED_NC = None


def kernel(x: np.ndarray, omega_kernel: np.ndarray, **run_kwargs) -> np.ndarray:
    global _CACHED_NC
    assert x.shape == (B, NJ, 1) and omega_kernel.shape == (1, 1), (
        x.shape,
        omega_kernel.shape,
    )
    x = np.ascontiguousarray(x, np.float32)
    om = np.ascontiguousarray(omega_kernel, np.float32)

    if _CACHED_NC is None:
        _CACHED_NC = _build()
    nc = _CACHED_NC

    in_maps = [
        {
            "x": x[c * BC : (c + 1) * BC].reshape(128, E),
            "om": om,
        }
        for c in range(N_CORES)
    ]
    res = run_bass_kernel_spmd(nc, in_maps, core_ids=list(range(N_CORES)), **run_kwargs)
    fg = TILE_F - FD
    full = np.empty((B, NJ, NJ), np.float32)
    for c in range(N_CORES):
        arr = np.asarray(res.results[c]["out"]).reshape(128, NJ * E)
        out3 = np.empty((128, NJ, E), np.float32)
        for it in range(E // TILE_F):
            f0, base = it * TILE_F, it * NJ * TILE_F
            out3[:, :, f0 : f0 + FD] = arr[
                :, base : base + NJ * FD
            ].reshape(128, NJ, FD)
            out3[:, :, f0 + FD : f0 + TILE_F] = arr[
                :, base + NJ * FD : base + NJ * TILE_F
            ].reshape(128, NJ, fg)
        full[c * BC : (c + 1) * BC] = out3.transpose(0, 2, 1).reshape(BC, NJ, NJ)
    if run_kwargs:
        return full, res
    return full



# revision 4
# speedup vs baseline: 1.1534x; 1.1534x over previous
"""Trainium2 Bass kernel for nn_HarmonicOscillatorOrbitals.

out[b, i, j] = exp(-s^2/2) * H_j(s), s = omega * x[b, i, 0], j = 0..31
(physicists' Hermite polynomials), data-parallel over 8 NeuronCores on
the leading batch axis.

Per core (8192 batches = 262144 scalars as [128 partitions, E=2048]):
  t   = 2*omega*x
  env = exp(-s^2/2) = exp(t^2 * -1/8)   (ACT Exp, ~2ulp)
  G_0 = env, G_1 = t*env, G_k = t*G_{k-1} - 2(k-1)*G_{k-2}  (= env*H_k)

Full-width column split: DVE owns cols [0, FD) (tensor_mul +
scalar_tensor_tensor, f32 1x), GPSIMD owns [FD, E) (tensor_mul +
scalar_tensor_tensor on Q7). ACT downcasts every G_k slice to bf16
staging tiles; groups of 4 k-levels are flushed to DRAM with one DMA
per slice (1.4MB / 0.7MB transfers). The f32 recurrence keeps rel err
~1e-6; the bf16 store adds ~2.6e-3 relative to the global max, well
under the 2e-2 gate. Host upcasts and permutes to (batch, i, j).
"""

from contextlib import ExitStack

import ml_dtypes
import numpy as np

import concourse.bacc as bacc
import concourse.mybir as mybir
import concourse.tile as tile
from concourse.bass_utils import run_bass_kernel_spmd

F32 = mybir.dt.float32
BF16 = mybir.dt.bfloat16
AF = mybir.ActivationFunctionType
ALU = mybir.AluOpType

NJ = 32          # number of Hermite orders
N_CORES = 8
B = 65536        # full batch
BC = B // N_CORES
E = BC * NJ // 128   # 2048 free elems per partition per core

FD = 1336        # DVE-owned columns (rest on GPSIMD)
KG = 4           # k-levels per DMA flush group


def _build(e=E, fd=FD, gps_stt=False):
    nc = bacc.Bacc("TRN2", target_bir_lowering=False, debug=False)
    x_d = nc.dram_tensor("x", [128, e], F32, kind="ExternalInput").ap()
    om_d = nc.dram_tensor("om", [1, 1], F32, kind="ExternalInput").ap()
    # raw dump: per group q of KG k-levels, [128, KG*fd] DVE slice then
    # [128, KG*fg] GPSIMD slice — host unscrambles
    out_d = nc.dram_tensor("out", [128, NJ * e], BF16, kind="ExternalOutput").ap()

    fg = e - fd
    n_groups = NJ // KG
    with tile.TileContext(nc) as tc, ExitStack() as ctx:
        cpool = ctx.enter_context(tc.tile_pool(name="const", bufs=1))
        xp = ctx.enter_context(tc.tile_pool(name="xp", bufs=1))
        qd = ctx.enter_context(tc.tile_pool(name="qd", bufs=3))
        qg = ctx.enter_context(tc.tile_pool(name="qg", bufs=3))
        gdp = ctx.enter_context(tc.tile_pool(name="gdp", bufs=4))
        ggp = ctx.enter_context(tc.tile_pool(name="ggp", bufs=4))
        sdp = ctx.enter_context(tc.tile_pool(name="sdp", bufs=2))
        sgp = ctx.enter_context(tc.tile_pool(name="sgp", bufs=2))

        om1 = cpool.tile([128, 1], F32)
        nc.sync.dma_start(om1[0:1, :], om_d[:, :])
        om2 = cpool.tile([128, 1], F32)
        nc.gpsimd.partition_broadcast(om2[:, :], om1[0:1, :])
        nc.scalar.mul(om2[:, :], om2[:, :], 2.0)  # om2 = 2*omega

        x_t = xp.tile([128, e], F32)
        nc.sync.dma_start(x_t[:, :], x_d[:, :])
        t_t = xp.tile([128, e], F32, name="t")
        nc.scalar.mul(t_t[:, :], x_t[:, :], om2[:, 0:1])  # t = 2*omega*x
        sq = xp.tile([128, e], F32, name="sq")
        nc.vector.tensor_mul(sq[:, :], t_t[:, :], t_t[:, :])  # t^2 = 4 s^2

        gd = []  # per-k DVE-slice f32 tiles (pool round-robins buffers)
        gg = []
        stage_d = [None] * n_groups
        stage_g = [None] * n_groups

        def downcast(k):
            q, r = divmod(k, KG)
            if r == 0:
                stage_d[q] = sdp.tile([128, KG * fd], BF16, name=f"sd{q}", tag="sd")
                stage_g[q] = sgp.tile([128, KG * fg], BF16, name=f"sg{q}", tag="sg")
            nc.scalar.copy(stage_d[q][:, r * fd : (r + 1) * fd], gd[k][:, :])
            nc.scalar.copy(stage_g[q][:, r * fg : (r + 1) * fg], gg[k][:, :])
            if r == KG - 1:
                base = q * KG * e
                nc.sync.dma_start(
                    out_d[:, base : base + KG * fd], stage_d[q][:, :]
                )
                nc.sync.dma_start(
                    out_d[:, base + KG * fd : base + KG * e], stage_g[q][:, :]
                )

        # k = 0: env = exp(-t^2/8) split straight into the two chains
        gd.append(gdp.tile([128, fd], F32, name="gd0", tag="gd"))
        gg.append(ggp.tile([128, fg], F32, name="gg0", tag="gg"))
        nc.scalar.activation(gd[0][:, :], sq[:, 0:fd], AF.Exp, scale=-0.125)
        nc.scalar.activation(gg[0][:, :], sq[:, fd:], AF.Exp, scale=-0.125)

        # k = 1: G_1 = t * env
        gd.append(gdp.tile([128, fd], F32, name="gd1", tag="gd"))
        gg.append(ggp.tile([128, fg], F32, name="gg1", tag="gg"))
        nc.vector.tensor_mul(gd[1][:, :], t_t[:, 0:fd], gd[0][:, :])
        nc.gpsimd.tensor_mul(gg[1][:, :], t_t[:, fd:], gg[0][:, :])
        downcast(0)
        downcast(1)

        for k in range(2, NJ):
            c = 2.0 * (k - 1)
            # DVE chain
            q_t = qd.tile([128, fd], F32)
            nc.vector.tensor_mul(q_t[:, :], t_t[:, 0:fd], gd[k - 1][:, :])
            gd.append(gdp.tile([128, fd], F32, name=f"gd{k}", tag="gd"))
            nc.vector.scalar_tensor_tensor(
                gd[k][:, :], gd[k - 2][:, :], -c, q_t[:, :], ALU.mult, ALU.add
            )
            # GPSIMD chain
            qg_t = qg.tile([128, fg], F32)
            nc.gpsimd.tensor_mul(qg_t[:, :], t_t[:, fd:], gg[k - 1][:, :])
            gg.append(ggp.tile([128, fg], F32, name=f"gg{k}", tag="gg"))
            if gps_stt:
                nc.gpsimd.scalar_tensor_tensor(
                    gg[k][:, :], gg[k - 2][:, :], -c, qg_t[:, :], ALU.mult, ALU.add
                )
            else:
                rg_t = qg.tile([128, fg], F32, name=f"rg{k}", tag="rg")
                nc.scalar.mul(rg_t[:, :], gg[k - 2][:, :], c)
                nc.gpsimd.tensor_sub(gg[k][:, :], qg_t[:, :], rg_t[:, :])
            downcast(k)

    nc.compile()
    return nc


_CACHED_NC = None


def kernel(x: np.ndarray, omega_kernel: np.ndarray, **run_kwargs) -> np.ndarray:
    global _CACHED_NC
    assert x.shape == (B, NJ, 1) and omega_kernel.shape == (1, 1), (
        x.shape,
        omega_kernel.shape,
    )
    x = np.ascontiguousarray(x, np.float32)
    om = np.ascontiguousarray(omega_kernel, np.float32)

    if _CACHED_NC is None:
        _CACHED_NC = _build()
    nc = _CACHED_NC

    in_maps = [
        {
            "x": x[c * BC : (c + 1) * BC].reshape(128, E),
            "om": om,
        }
        for c in range(N_CORES)
    ]
    res = run_bass_kernel_spmd(nc, in_maps, core_ids=list(range(N_CORES)), **run_kwargs)
    fg = E - FD
    full = np.empty((B, NJ, NJ), np.float32)
    out3 = np.empty((128, NJ, E), np.float32)
    for c in range(N_CORES):
        arr = np.asarray(res.results[c]["out"]).reshape(128, NJ * E)
        for q in range(NJ // KG):
            base = q * KG * E
            out3[:, q * KG : (q + 1) * KG, 0:FD] = (
                arr[:, base : base + KG * FD].astype(np.float32).reshape(128, KG, FD)
            )
            out3[:, q * KG : (q + 1) * KG, FD:] = (
                arr[:, base + KG * FD : base + KG * E]
                .astype(np.float32)
                .reshape(128, KG, fg)
            )
        full[c * BC : (c + 1) * BC] = (
            out3.transpose(0, 2, 1).reshape(BC, NJ, NJ)
        )
    if run_kwargs:
        return full, res
    return full


# revision 5
# speedup vs baseline: 2.0389x; 1.7677x over previous
"""Trainium2 Bass kernel for nn_HarmonicOscillatorOrbitals.

out[b, i, j] = exp(-s^2/2) * H_j(s), s = omega * x[b, i, 0], j = 0..31
(physicists' Hermite polynomials), data-parallel over 8 NeuronCores on
the leading batch axis.

Per core (8192 batches = 262144 scalars as [128 partitions, E=2048]),
the G_k = env*H_k recurrence runs entirely on DVE in *scaled fp16*:
each level is stored as Gh_k = G_k / 2^{e_k} (e_k = per-level
amplitude exponents, hardcoded), which keeps values in fp16 range and
makes every multiplier a power of two:

  q_k  = (t_hat / 2^{d_k}) * Gh_{k-1}     DVE tensor_tensor fp16, 2x mode
  r_k  = -b_k * Gh_{k-2}                  ACT scale-copy (own SBUF port)
  Gh_k = q_k + r_k                        DVE tensor_tensor fp16, 2x mode

t_hat = fp16(2*omega*x); its rounding error is repaired on the final
level with a first-order t_lo correction (dG/dt = k*G_{k-1} -
(t/4)*G_k), which brings max rel err to ~1.0e-2 of the global max
(gate 2e-2) — verified bit-exact against a numpy model of DVE's
fp32-internal/round-to-nearest-fp16 behavior.

GPSIMD is intentionally idle: it shares its SBUF port pair with DVE
(exclusive per-instruction lock), so any GPSIMD tensor work would
serialize against the DVE chain at worse per-element efficiency.

Output: fp16 scaled levels, level-major [128, NJ, E]; groups of 4
levels per 2MB DMA. Host multiplies by 2^{e_k}, upcasts to f32 and
permutes to (batch, i, j).
"""

from contextlib import ExitStack

import numpy as np

import concourse.bacc as bacc
import concourse.mybir as mybir
import concourse.tile as tile
from concourse.bass_utils import run_bass_kernel_spmd

F32 = mybir.dt.float32
F16 = mybir.dt.float16
AF = mybir.ActivationFunctionType
ALU = mybir.AluOpType

NJ = 32          # number of Hermite orders
N_CORES = 8
B = 65536        # full batch
BC = B // N_CORES
E = BC * NJ // 128   # 2048 free elems per partition per core

KG = 4           # k-levels per DMA flush group

# Per-level scale exponents e_k: Gh_k = G_k / 2^{e_k}.  amp_k =
# max_{|s|<=5.1} |env*H_k| computed offline in float64, e_k = ceil(log2).
EXP = [0, 1, 2, 3, 4, 6, 8, 10, 12, 14, 16, 18, 20, 23, 25, 28,
       30, 33, 35, 37, 40, 43, 46, 48, 51, 54, 57, 59, 62, 65, 68, 71]
SIG = [float(2.0**e) for e in EXP]


def _build(e=E):
    nc = bacc.Bacc("TRN2", target_bir_lowering=False, debug=False)
    x_d = nc.dram_tensor("x", [128, e], F32, kind="ExternalInput").ap()
    om_d = nc.dram_tensor("om", [1, 1], F32, kind="ExternalInput").ap()
    out_d = nc.dram_tensor("out", [128, NJ * e], F16, kind="ExternalOutput").ap()

    n_groups = NJ // KG
    with tile.TileContext(nc) as tc, ExitStack() as ctx:
        cpool = ctx.enter_context(tc.tile_pool(name="const", bufs=1))
        xp = ctx.enter_context(tc.tile_pool(name="xp", bufs=1))
        qp = ctx.enter_context(tc.tile_pool(name="qp", bufs=3))
        rp = ctx.enter_context(tc.tile_pool(name="rp", bufs=3))
        gp = ctx.enter_context(tc.tile_pool(name="gp", bufs=3))

        om1 = cpool.tile([128, 1], F32)
        nc.sync.dma_start(om1[0:1, :], om_d[:, :])
        om2 = cpool.tile([128, 1], F32)
        nc.gpsimd.partition_broadcast(om2[:, :], om1[0:1, :])
        nc.scalar.mul(om2[:, :], om2[:, :], 2.0)  # om2 = 2*omega

        x_t = xp.tile([128, e], F32)
        nc.sync.dma_start(x_t[:, :], x_d[:, :])
        t_t = xp.tile([128, e], F32, name="t")
        nc.scalar.mul(t_t[:, :], x_t[:, :], om2[:, 0:1])  # t = 2*omega*x
        th = xp.tile([128, e], F16, name="th")
        nc.scalar.copy(th[:, :], t_t[:, :])               # t_hat = fp16(t)
        tl = xp.tile([128, e], F16, name="tl")
        nc.vector.scalar_tensor_tensor(                   # t_lo = t - t_hat
            tl[:, :], th[:, :], -1.0, t_t[:, :], ALU.mult, ALU.add
        )
        # pre-scaled t_hat variants: th/2^d (fp16 halvings, exact)
        ta = {}
        for dd in (1, 2, 3):
            ta[dd] = xp.tile([128, e], F16, name=f"ta{dd}")
            nc.vector.tensor_scalar_mul(ta[dd][:, :], th[:, :], 0.5**dd)
        # w1 = -(t_hat/4)*t_lo for the final-level correction
        w1 = xp.tile([128, e], F16, name="w1")
        nc.vector.scalar_tensor_tensor(
            w1[:, :], th[:, :], -0.25, tl[:, :], ALU.mult, ALU.mult
        )
        sq = xp.tile([128, e], F32, name="sq")
        nc.vector.tensor_mul(sq[:, :], t_t[:, :], t_t[:, :])  # t^2

        # level-group tiles: [128, KG*e] fp16, level k in slice k%KG
        groups = [None] * n_groups

        def gh(k):
            return groups[k // KG][:, (k % KG) * e : (k % KG + 1) * e]

        def open_group(k):
            q, r = divmod(k, KG)
            if r == 0:
                groups[q] = gp.tile([128, KG * e], F16, name=f"grp{q}", tag="g")

        def flush_group(k):
            q, r = divmod(k, KG)
            if r == KG - 1:
                nc.sync.dma_start(
                    out_d[:, q * KG * e : (q + 1) * KG * e], groups[q][:, :]
                )

        # Gh_0 = env = exp(-t^2/8)
        open_group(0)
        nc.scalar.activation(gh(0), sq[:, :], AF.Exp, scale=-0.125)
        # Gh_1 = (t/2)*env  (sigma_1 = 2)
        nc.vector.scalar_tensor_tensor(
            gh(1), t_t[:, :], 0.5, gh(0), ALU.mult, ALU.mult
        )

        for k in range(2, NJ):
            open_group(k)
            dk = EXP[k] - EXP[k - 1]
            b = 2.0 * (k - 1) * SIG[k - 2] / SIG[k]
            q_t = qp.tile([128, e], F16, name=f"q{k}", tag="q")
            nc.vector.tensor_mul(q_t[:, :], ta[dk][:, :], gh(k - 1))
            r_t = rp.tile([128, e], F16, name=f"r{k}", tag="r")
            nc.scalar.mul(r_t[:, :], gh(k - 2), -b)
            nc.vector.tensor_add(gh(k), q_t[:, :], r_t[:, :])
            if k < NJ - 1:
                flush_group(k)

        # final-level correction: Gh_31 += t_lo*(31*(sig30/sig31)*Gh_30
        #                                        - (t_hat/4)*Gh_31)
        c1 = NJ - 1.0
        c1 = float(c1 * SIG[NJ - 2] / SIG[NJ - 1])
        z1a = rp.tile([128, e], F16, name="z1a", tag="r")
        nc.vector.tensor_scalar_mul(z1a[:, :], tl[:, :], c1)
        z1 = qp.tile([128, e], F16, name="z1", tag="q")
        nc.vector.tensor_mul(z1[:, :], z1a[:, :], gh(NJ - 2))
        z2 = qp.tile([128, e], F16, name="z2", tag="q")
        nc.vector.tensor_mul(z2[:, :], w1[:, :], gh(NJ - 1))
        s_t = qp.tile([128, e], F16, name="s", tag="q")
        nc.vector.tensor_add(s_t[:, :], z1[:, :], z2[:, :])
        nc.vector.tensor_add(gh(NJ - 1), s_t[:, :], gh(NJ - 1))
        flush_group(NJ - 1)

    nc.compile()
    return nc


_CACHED_NC = None


def kernel(x: np.ndarray, omega_kernel: np.ndarray, **run_kwargs) -> np.ndarray:
    global _CACHED_NC
    assert x.shape == (B, NJ, 1) and omega_kernel.shape == (1, 1), (
        x.shape,
        omega_kernel.shape,
    )
    x = np.ascontiguousarray(x, np.float32)
    om = np.ascontiguousarray(omega_kernel, np.float32)

    if _CACHED_NC is None:
        _CACHED_NC = _build()
    nc = _CACHED_NC

    in_maps = [
        {
            "x": x[c * BC : (c + 1) * BC].reshape(128, E),
            "om": om,
        }
        for c in range(N_CORES)
    ]
    res = run_bass_kernel_spmd(nc, in_maps, core_ids=list(range(N_CORES)), **run_kwargs)
    sig = np.asarray(SIG, np.float32)  # [NJ]
    full = np.empty((B, NJ, NJ), np.float32)
    for c in range(N_CORES):
        arr = np.asarray(res.results[c]["out"]).reshape(128, NJ, E)
        out3 = arr.astype(np.float32) * sig[None, :, None]
        # scalar index = p*E + col ; out[b, i, j] = out3[p, j, col]
        full[c * BC : (c + 1) * BC] = (
            out3.transpose(0, 2, 1).reshape(BC, NJ, NJ)
        )
    if run_kwargs:
        return full, res
    return full


# revision 7
# speedup vs baseline: 2.0772x; 1.0188x over previous
"""Trainium2 Bass kernel for nn_HarmonicOscillatorOrbitals.

out[b, i, j] = exp(-s^2/2) * H_j(s), s = omega * x[b, i, 0], j = 0..31
(physicists' Hermite polynomials), data-parallel over 8 NeuronCores on
the leading batch axis.

Per core (8192 batches = 262144 scalars as [128 partitions, E=2048]),
the G_k = env*H_k recurrence runs entirely on DVE in *scaled fp16*:
each level is stored as Gh_k = G_k / 2^{e_k} (e_k = per-level
amplitude exponents, hardcoded), which keeps values in fp16 range and
makes every multiplier a power of two:

  q_k  = (t_hat / 2^{d_k}) * Gh_{k-1}     DVE tensor_tensor fp16, 2x mode
  r_k  = -b_k * Gh_{k-2}                  ACT scale-copy (own SBUF port)
  Gh_k = q_k + r_k                        DVE tensor_tensor fp16, 2x mode

t_hat = fp16(2*omega*x); its rounding error is repaired on the final
level with a first-order t_lo correction (dG/dt = k*G_{k-1} -
(t/4)*G_k), which brings max rel err to ~1.05e-2 of the global max
(gate 2e-2) — verified bit-exact against a numpy model of DVE's
fp32-internal/round-to-nearest-fp16 behavior.

GPSIMD is intentionally idle: it shares its SBUF port pair with DVE
(exclusive per-instruction lock), so any GPSIMD tensor work would
serialize against the DVE chain at worse per-element efficiency.

Startup is pipelined in two column halves (x DMA -> t -> sq/th ->
env -> Gh_1 per half) so the chain starts ~13us in. Output: fp16
scaled levels, level-major [128, NJ, E]; 4-level 2MB DMA groups (the
last group flushes as 2+2 levels around the correction). Host
multiplies by 2^{e_k}, upcasts to f32 and permutes to (batch, i, j).
"""

from contextlib import ExitStack

import numpy as np

import concourse.bacc as bacc
import concourse.mybir as mybir
import concourse.tile as tile
from concourse.bass_utils import run_bass_kernel_spmd

F32 = mybir.dt.float32
F16 = mybir.dt.float16
AF = mybir.ActivationFunctionType
ALU = mybir.AluOpType

NJ = 32          # number of Hermite orders
N_CORES = 8
B = 65536        # full batch
BC = B // N_CORES
E = BC * NJ // 128   # 2048 free elems per partition per core

KG = 4           # k-levels per DMA flush group

# Per-level scale exponents e_k: Gh_k = G_k / 2^{e_k}.  amp_k =
# max_{|s|<=5.1} |env*H_k| computed offline in float64, e_k = ceil(log2).
EXP = [0, 1, 2, 3, 4, 6, 8, 10, 12, 14, 16, 18, 20, 23, 25, 28,
       30, 33, 35, 37, 40, 43, 46, 48, 51, 54, 57, 59, 62, 65, 68, 71]
SIG = [float(2.0**e) for e in EXP]


def _build(e=E):
    nc = bacc.Bacc("TRN2", target_bir_lowering=False, debug=False)
    x_d = nc.dram_tensor("x", [128, e], F32, kind="ExternalInput").ap()
    om_d = nc.dram_tensor("om", [1, 1], F32, kind="ExternalInput").ap()
    out_d = nc.dram_tensor("out", [128, NJ * e], F16, kind="ExternalOutput").ap()

    n_groups = NJ // KG
    h = e // 2
    halves = [(0, h), (h, e)]
    with tile.TileContext(nc) as tc, ExitStack() as ctx:
        cpool = ctx.enter_context(tc.tile_pool(name="const", bufs=1))
        xp = ctx.enter_context(tc.tile_pool(name="xp", bufs=1))
        qp = ctx.enter_context(tc.tile_pool(name="qp", bufs=3))
        rp = ctx.enter_context(tc.tile_pool(name="rp", bufs=3))
        gp = ctx.enter_context(tc.tile_pool(name="gp", bufs=3))
        zp = ctx.enter_context(tc.tile_pool(name="zp", bufs=3))

        om1 = cpool.tile([128, 1], F32)
        nc.sync.dma_start(om1[0:1, :], om_d[:, :])
        om2 = cpool.tile([128, 1], F32)
        nc.gpsimd.partition_broadcast(om2[:, :], om1[0:1, :])
        nc.scalar.mul(om2[:, :], om2[:, :], 2.0)  # om2 = 2*omega

        # level-group tiles: [128, KG*e] fp16, level k in slice k%KG
        groups = [None] * n_groups

        def gh(k):
            return groups[k // KG][:, (k % KG) * e : (k % KG + 1) * e]

        def open_group(k):
            q, r = divmod(k, KG)
            if r == 0:
                groups[q] = gp.tile([128, KG * e], F16, name=f"grp{q}", tag="g")

        # ---- two-half pipelined startup ----
        x_t = xp.tile([128, e], F32)
        t_t = xp.tile([128, e], F32, name="t")
        th = xp.tile([128, e], F16, name="th")
        sq = xp.tile([128, e], F32, name="sq")
        open_group(0)
        open_group(1)
        for lo, hi in halves:
            nc.sync.dma_start(x_t[:, lo:hi], x_d[:, lo:hi])
            nc.scalar.mul(t_t[:, lo:hi], x_t[:, lo:hi], om2[:, 0:1])
            nc.vector.tensor_mul(sq[:, lo:hi], t_t[:, lo:hi], t_t[:, lo:hi])
            nc.scalar.activation(
                groups[0][:, lo:hi], sq[:, lo:hi], AF.Exp, scale=-0.125
            )  # Gh_0 = env = exp(-t^2/8)
            nc.scalar.copy(th[:, lo:hi], t_t[:, lo:hi])  # t_hat = fp16(t)
            # Gh_1 = (t/2)*env  (sigma_1 = 2)
            nc.vector.scalar_tensor_tensor(
                groups[0][:, e + lo : e + hi],
                t_t[:, lo:hi],
                0.5,
                groups[0][:, lo:hi],
                ALU.mult,
                ALU.mult,
            )

        # pre-scaled t_hat variants: th/2^d (fp16 halvings, exact)
        ta = {}
        for dd in (1, 2, 3):
            ta[dd] = xp.tile([128, e], F16, name=f"ta{dd}")
            nc.vector.tensor_scalar_mul(ta[dd][:, :], th[:, :], 0.5**dd)
        # correction prep (runs in the pre-chain DVE idle window):
        # t_lo = t - t_hat ; w1 = -(t_hat/4)*t_lo ; z1a = c1*t_lo
        tl = xp.tile([128, e], F16, name="tl")
        nc.vector.scalar_tensor_tensor(
            tl[:, :], th[:, :], -1.0, t_t[:, :], ALU.mult, ALU.add
        )
        w1p = xp.tile([128, e], F16, name="w1p")
        nc.vector.tensor_mul(w1p[:, :], th[:, :], tl[:, :])
        w1 = xp.tile([128, e], F16, name="w1")
        nc.vector.tensor_scalar_mul(w1[:, :], w1p[:, :], -0.25)
        c1 = float((NJ - 1.0) * SIG[NJ - 2] / SIG[NJ - 1])
        z1a = xp.tile([128, e], F16, name="z1a")
        nc.vector.tensor_scalar_mul(z1a[:, :], tl[:, :], c1)

        def flush(k0, k1):  # DMA levels [k0, k1] (same group) to DRAM
            q = k0 // KG
            r0, r1 = k0 % KG, k1 % KG
            nc.sync.dma_start(
                out_d[:, k0 * e : (k1 + 1) * e],
                groups[q][:, r0 * e : (r1 + 1) * e],
            )

        z1 = None
        for k in range(2, NJ):
            open_group(k)
            dk = EXP[k] - EXP[k - 1]
            b = 2.0 * (k - 1) * SIG[k - 2] / SIG[k]
            q_t = qp.tile([128, e], F16, name=f"q{k}", tag="q")
            nc.vector.tensor_mul(q_t[:, :], ta[dk][:, :], gh(k - 1))
            r_t = rp.tile([128, e], F16, name=f"r{k}", tag="r")
            nc.scalar.mul(r_t[:, :], gh(k - 2), -b)
            nc.vector.tensor_add(gh(k), q_t[:, :], r_t[:, :])
            if k % KG == KG - 1 and k < NJ - 1:
                flush(k - 3, k)
            if k == NJ - 2:
                # z1 = (c1*t_lo)*Gh_30 — issue before the last level's ops
                z1 = zp.tile([128, e], F16, name="z1", tag="z")
                nc.vector.tensor_mul(z1[:, :], z1a[:, :], gh(k))

        # final-level correction: Gh_31 += t_lo*(31*(sig30/sig31)*Gh_30
        #                                        - (t_hat/4)*Gh_31)
        flush(NJ - 4, NJ - 3)
        z2 = zp.tile([128, e], F16, name="z2", tag="z")
        nc.vector.tensor_mul(z2[:, :], w1[:, :], gh(NJ - 1))
        s_t = zp.tile([128, e], F16, name="s", tag="z")
        nc.vector.tensor_add(s_t[:, :], z1[:, :], z2[:, :])
        nc.vector.tensor_add(gh(NJ - 1), s_t[:, :], gh(NJ - 1))
        flush(NJ - 2, NJ - 1)

    nc.compile()
    return nc


_CACHED_NC = None


def kernel(x: np.ndarray, omega_kernel: np.ndarray, **run_kwargs) -> np.ndarray:
    global _CACHED_NC
    assert x.shape == (B, NJ, 1) and omega_kernel.shape == (1, 1), (
        x.shape,
        omega_kernel.shape,
    )
    x = np.ascontiguousarray(x, np.float32)
    om = np.ascontiguousarray(omega_kernel, np.float32)

    if _CACHED_NC is None:
        _CACHED_NC = _build()
    nc = _CACHED_NC

    in_maps = [
        {
            "x": x[c * BC : (c + 1) * BC].reshape(128, E),
            "om": om,
        }
        for c in range(N_CORES)
    ]
    res = run_bass_kernel_spmd(nc, in_maps, core_ids=list(range(N_CORES)), **run_kwargs)
    sig = np.asarray(SIG, np.float32)  # [NJ]
    full = np.empty((B, NJ, NJ), np.float32)
    for c in range(N_CORES):
        arr = np.asarray(res.results[c]["out"]).reshape(128, NJ, E)
        out3 = arr.astype(np.float32) * sig[None, :, None]
        # scalar index = p*E + col ; out[b, i, j] = out3[p, j, col]
        full[c * BC : (c + 1) * BC] = (
            out3.transpose(0, 2, 1).reshape(BC, NJ, NJ)
        )
    if run_kwargs:
        return full, res
    return full


# revision 8
# speedup vs baseline: 2.2457x; 1.0811x over previous
"""Trainium2 Bass kernel for nn_HarmonicOscillatorOrbitals.

out[b, i, j] = exp(-s^2/2) * H_j(s), s = omega * x[b, i, 0], j = 0..31
(physicists' Hermite polynomials), data-parallel over 8 NeuronCores on
the leading batch axis.

Per core (8192 batches = 262144 scalars as [128 partitions, E=2048]),
the G_k = env*H_k recurrence runs entirely on DVE in *scaled fp16*:
each level is stored as Gh_k = G_k / 2^{e_k} (e_k = per-level
amplitude exponents, hardcoded), which keeps values in fp16 range and
makes every multiplier a power of two:

  q_k  = (t_hat / 2^{d_k}) * Gh_{k-1}     DVE tensor_tensor fp16, 2x mode
  r_k  = -b_k * Gh_{k-2}                  ACT scale-copy (own SBUF port)
  Gh_k = q_k + r_k                        DVE tensor_tensor fp16, 2x mode

t_hat = fp16(2*omega*x); its rounding error is repaired on the final
level with a first-order t_lo correction (dG/dt = k*G_{k-1} -
(t/4)*G_k), which brings max rel err to ~1.05e-2 of the global max
(gate 2e-2) — verified bit-exact against a numpy model of DVE's
fp32-internal/round-to-nearest-fp16 behavior.

GPSIMD is intentionally idle: it shares its SBUF port pair with DVE
(exclusive per-instruction lock), so any GPSIMD tensor work would
serialize against the DVE chain at worse per-element efficiency.

Startup is pipelined in two column halves (x DMA -> t -> sq/th ->
env -> Gh_1 per half) so the chain starts ~13us in. Output: fp16
scaled levels, level-major [128, NJ, E]; 4-level 2MB DMA groups (the
last group flushes as 2+2 levels around the correction). Host
multiplies by 2^{e_k}, upcasts to f32 and permutes to (batch, i, j).
"""

from contextlib import ExitStack

import numpy as np

import concourse.bacc as bacc
import concourse.mybir as mybir
import concourse.tile as tile
from concourse.bass_utils import run_bass_kernel_spmd

F32 = mybir.dt.float32
F16 = mybir.dt.float16
AF = mybir.ActivationFunctionType
ALU = mybir.AluOpType

NJ = 32          # number of Hermite orders
N_CORES = 8
B = 65536        # full batch
BC = B // N_CORES
E = BC * NJ // 128   # 2048 free elems per partition per core

KG = 4           # k-levels per DMA flush group

# Per-level scale exponents e_k: Gh_k = G_k / 2^{e_k}.  amp_k =
# max_{|s|<=5.1} |env*H_k| computed offline in float64, e_k = ceil(log2).
EXP = [0, 1, 2, 3, 4, 6, 8, 10, 12, 14, 16, 18, 20, 23, 25, 28,
       30, 33, 35, 37, 40, 43, 46, 48, 51, 54, 57, 59, 62, 65, 68, 71]
SIG = [float(2.0**e) for e in EXP]


def _build(e=E):
    nc = bacc.Bacc("TRN2", target_bir_lowering=False, debug=False)
    x_d = nc.dram_tensor("x", [128, e], F32, kind="ExternalInput").ap()
    om_d = nc.dram_tensor("om", [128, 1], F32, kind="ExternalInput").ap()
    out_d = nc.dram_tensor("out", [128, NJ * e], F16, kind="ExternalOutput").ap()

    n_groups = NJ // KG
    h = e // 2
    halves = [(0, h), (h, e)]
    with tile.TileContext(nc) as tc, ExitStack() as ctx:
        cpool = ctx.enter_context(tc.tile_pool(name="const", bufs=1))
        xp = ctx.enter_context(tc.tile_pool(name="xp", bufs=1))
        qp = ctx.enter_context(tc.tile_pool(name="qp", bufs=3))
        rp = ctx.enter_context(tc.tile_pool(name="rp", bufs=3))
        gp = ctx.enter_context(tc.tile_pool(name="gp", bufs=3))
        zp = ctx.enter_context(tc.tile_pool(name="zp", bufs=3))

        om2 = cpool.tile([128, 1], F32)
        nc.sync.dma_start(om2[:, :], om_d[:, :])
        nc.scalar.mul(om2[:, :], om2[:, :], 2.0)  # om2 = 2*omega

        # level-group tiles: [128, KG*e] fp16, level k in slice k%KG
        groups = [None] * n_groups

        def gh(k):
            return groups[k // KG][:, (k % KG) * e : (k % KG + 1) * e]

        def open_group(k):
            q, r = divmod(k, KG)
            if r == 0:
                groups[q] = gp.tile([128, KG * e], F16, name=f"grp{q}", tag="g")

        # ---- two-half pipelined startup ----
        x_t = xp.tile([128, e], F32)
        t_t = xp.tile([128, e], F32, name="t")
        th = xp.tile([128, e], F16, name="th")
        sq = xp.tile([128, e], F32, name="sq")
        open_group(0)
        open_group(1)
        for lo, hi in halves:
            nc.sync.dma_start(x_t[:, lo:hi], x_d[:, lo:hi])
            nc.scalar.mul(t_t[:, lo:hi], x_t[:, lo:hi], om2[:, 0:1])
            nc.vector.tensor_mul(sq[:, lo:hi], t_t[:, lo:hi], t_t[:, lo:hi])
            nc.scalar.activation(
                groups[0][:, lo:hi], sq[:, lo:hi], AF.Exp, scale=-0.125
            )  # Gh_0 = env = exp(-t^2/8)
            nc.scalar.copy(th[:, lo:hi], t_t[:, lo:hi])  # t_hat = fp16(t)
            # Gh_1 = (t/2)*env  (sigma_1 = 2)
            nc.vector.scalar_tensor_tensor(
                groups[0][:, e + lo : e + hi],
                t_t[:, lo:hi],
                0.5,
                groups[0][:, lo:hi],
                ALU.mult,
                ALU.mult,
            )

        # pre-scaled t_hat variants: th/2^d (fp16 halvings, exact)
        ta = {}
        for dd in (1, 2, 3):
            ta[dd] = xp.tile([128, e], F16, name=f"ta{dd}")
            nc.vector.tensor_scalar_mul(ta[dd][:, :], th[:, :], 0.5**dd)
        # correction prep (runs in the pre-chain DVE idle window):
        # t_lo = t - t_hat ; w1 = -(t_hat/4)*t_lo ; z1a = c1*t_lo
        tl = xp.tile([128, e], F16, name="tl")
        nc.vector.scalar_tensor_tensor(
            tl[:, :], th[:, :], -1.0, t_t[:, :], ALU.mult, ALU.add
        )
        w1p = xp.tile([128, e], F16, name="w1p")
        nc.vector.tensor_mul(w1p[:, :], th[:, :], tl[:, :])
        w1x = xp.tile([128, e], F16, name="w1x")  # 1 - (t_hat/4)*t_lo
        nc.scalar.activation(w1x[:, :], w1p[:, :], AF.Copy, bias=1.0, scale=-0.25)
        c1 = float((NJ - 1.0) * SIG[NJ - 2] / SIG[NJ - 1])
        z1a = xp.tile([128, e], F16, name="z1a")
        nc.scalar.mul(z1a[:, :], tl[:, :], c1)

        def flush(k0, k1):  # DMA levels [k0, k1] (same group) to DRAM
            q = k0 // KG
            r0, r1 = k0 % KG, k1 % KG
            nc.sync.dma_start(
                out_d[:, k0 * e : (k1 + 1) * e],
                groups[q][:, r0 * e : (r1 + 1) * e],
            )

        z1 = None
        for k in range(2, NJ):
            open_group(k)
            dk = EXP[k] - EXP[k - 1]
            b = 2.0 * (k - 1) * SIG[k - 2] / SIG[k]
            q_t = qp.tile([128, e], F16, name=f"q{k}", tag="q")
            nc.vector.tensor_mul(q_t[:, :], ta[dk][:, :], gh(k - 1))
            r_t = rp.tile([128, e], F16, name=f"r{k}", tag="r")
            nc.scalar.mul(r_t[:, :], gh(k - 2), -b)
            if k == NJ - 1:
                break
            nc.vector.tensor_add(gh(k), q_t[:, :], r_t[:, :])
            if k % KG == KG - 1 and k < NJ - 1:
                flush(k - 3, k)
            if k == NJ - 2:
                flush(k - 2, k - 1)  # 28, 29
                # z1 = (c1*t_lo)*Gh_30 — issue before the last level's ops
                z1 = zp.tile([128, e], F16, name="z1", tag="z")
                nc.vector.tensor_mul(z1[:, :], z1a[:, :], gh(k))
                flush(k, k)  # 30

        # final level with folded t_lo correction:
        #   u = q + r (uncorrected Gh_31); Gh_31 = u*(1 - (t_hat/4)*t_lo) + z1
        u_t = zp.tile([128, e], F16, name="u", tag="z")
        nc.vector.tensor_add(u_t[:, :], q_t[:, :], r_t[:, :])
        v_t = zp.tile([128, e], F16, name="v", tag="z")
        nc.vector.tensor_mul(v_t[:, :], u_t[:, :], w1x[:, :])
        nc.vector.tensor_add(gh(NJ - 1), v_t[:, :], z1[:, :])
        flush(NJ - 1, NJ - 1)

    nc.compile()
    return nc


_CACHED_NC = None


def kernel(x: np.ndarray, omega_kernel: np.ndarray, **run_kwargs) -> np.ndarray:
    global _CACHED_NC
    assert x.shape == (B, NJ, 1) and omega_kernel.shape == (1, 1), (
        x.shape,
        omega_kernel.shape,
    )
    x = np.ascontiguousarray(x, np.float32)
    om = np.ascontiguousarray(omega_kernel, np.float32)

    if _CACHED_NC is None:
        _CACHED_NC = _build()
    nc = _CACHED_NC

    in_maps = [
        {
            "x": x[c * BC : (c + 1) * BC].reshape(128, E),
            "om": np.ascontiguousarray(np.broadcast_to(om, (128, 1))),
        }
        for c in range(N_CORES)
    ]
    res = run_bass_kernel_spmd(nc, in_maps, core_ids=list(range(N_CORES)), **run_kwargs)
    sig = np.asarray(SIG, np.float32)  # [NJ]
    full = np.empty((B, NJ, NJ), np.float32)
    for c in range(N_CORES):
        arr = np.asarray(res.results[c]["out"]).reshape(128, NJ, E)
        out3 = arr.astype(np.float32) * sig[None, :, None]
        # scalar index = p*E + col ; out[b, i, j] = out3[p, j, col]
        full[c * BC : (c + 1) * BC] = (
            out3.transpose(0, 2, 1).reshape(BC, NJ, NJ)
        )
    if run_kwargs:
        return full, res
    return full
